# revision 1
# baseline (speedup 1.0000x reference)
"""BigBird simulated attention on 8 Trainium2 NeuronCores.

Strategy
--------
B*H = 24 (batch, head) pairs are sharded 3-per-core across 8 cores (data/head
parallel). The BigBird mask is block-constant on 64x64 tiles, so the host
compresses it to a 64x64 block map and bakes a block-sparse schedule directly
into the instruction stream (the mask never goes to the device).

Per (head, q-block of 64 rows) scores are computed TRANSPOSED (S^T: k on
partitions, q on free) so the exp'd probabilities are directly the stationary
operand of the PV matmul -- no on-chip transposes:

  S^T[k, q] = sum_d K[k, d] Q[q, d]    (lhsT = K^T block cols, rhs = Q^T)
  P^T = exp(S^T / 8)                    (one ScalarE activation per wave)
  acc[q, :] = sum_k P^T[k, q]^T Vaug[k, :]    with Vaug = [V | 1]

The ones-column of Vaug makes acc[:, 64] the softmax denominator, so the
normalization is one reciprocal + per-partition-scalar multiply at the end.
Max-subtraction is skipped: scores are ~N(0,1) after scaling, exp cannot
overflow, and softmax is shift-invariant.

Attended k-blocks are packed two-per-"chunk" (one 128-partition score tile
column group). Aligned pairs (2t, 2t+1) come straight from a resident packed
V ([128, 32*65], natural 128-row chunks); arbitrary pairs use a host-gathered
per-chunk V-pair tensor streamed per wave. Every PV matmul is then a uniform
128-contraction. All QK matmuls keep lhsT in PE row group 0-1 (lower 64
partitions) and slice the output partition range via column tiling instead --
mixing lower- and upper-row-group LDWEIGHTS hangs the PE at runtime on this
toolchain (bisected empirically).

The many 1x1 "observer" matmuls / nops exist because every engine instruction
on this toolchain carries at most ONE hardware sync wait: each observer
brings one engine up to date with one foreign semaphore so no real
instruction ever needs two waits.
"""

import numpy as np

import concourse.bass as bass
import concourse.tile as tile
from concourse import mybir
from concourse.bass_utils import run_bass_kernel_spmd
from concourse.tile_rust import add_dep_helper

B, H, S, D = 2, 12, 4096, 64
BLK = 64
NB = S // BLK            # 64 blocks per axis
DA = D + 1               # v plus ones column
NCORES = 8
HPC = B * H // NCORES    # heads per core
SCALE = 1.0 / 8.0        # 1/sqrt(64)
WAVE_CHUNKS = 23         # 23*64 cols used of a 3-bank score tile; tail reserved
OPB = 7                  # q-blocks per psum output bank (7*65 = 455 <= 512)
NBANK = (NB + OPB - 1) // OPB
NCHUNK = S // 128        # natural 128-row chunks of V

F32 = mybir.dt.float32


# ----------------------------------------------------------------- schedule

def _block_mask(mask: np.ndarray) -> np.ndarray:
    m = np.asarray(mask).reshape(NB, BLK, NB, BLK)
    bm = m[:, 0, :, 0]
    assert bool(np.all(m == bm[:, None, :, None])), (
        "mask is not 64x64 block-constant; this kernel's schedule requires it"
    )
    return bm > 0


def _qblock_chunks(row: np.ndarray):
    """Chunks for one q-block; each chunk = 2 block slots (128 partitions).

    ('P', t, None): aligned pair (2t, 2t+1) -- V comes from the resident
        packed tensor.
    ('S', gA, gB): arbitrary pair (either may be None) -- V comes from the
        host-gathered per-chunk pair tensor.
    """
    L = set(np.nonzero(row)[0].tolist())
    chunks, singles = [], []
    for t in range(NB // 2):
        a, b = 2 * t, 2 * t + 1
        if a in L and b in L:
            chunks.append(("P", t, None))
        else:
            if a in L:
                singles.append(a)
            if b in L:
                singles.append(b)
    for k in range(0, len(singles), 2):
        gA = singles[k]
        gB = singles[k + 1] if k + 1 < len(singles) else None
        chunks.append(("S", gA, gB))
    cov = []
    for c in chunks:
        if c[0] == "P":
            cov += [2 * c[1], 2 * c[1] + 1]
        else:
            cov += [g for g in c[1:] if g is not None]
    assert sorted(cov) == sorted(L)
    return chunks


def _build_schedule(bm: np.ndarray):
    """Flat chunk list [(i, chunk, first_of_i, last_of_i, s_idx)] in waves.

    s_idx: running index into the packed S-chunk V-pair tensor (or -1).
    """
    flat = []
    ns = 0
    for i in range(NB):
        chunks = _qblock_chunks(bm[i])
        assert chunks, f"q-block {i} attends to nothing"
        for ci, ch in enumerate(chunks):
            sidx = -1
            if ch[0] == "S":
                sidx = ns
                ns += 1
            flat.append((i, ch, ci == 0, ci == len(chunks) - 1, sidx))
    waves = [flat[o: o + WAVE_CHUNKS] for o in range(0, len(flat), WAVE_CHUNKS)]
    return waves, ns


# ------------------------------------------------------------------ program

def _crumb_cell2(rec, crumb):
    c = rec["ncr"]
    rec["ncr"] += 1
    assert rec["ncr"] <= 512
    return crumb[c // 512: c // 512 + 1, c % 512: c % 512 + 1]


def _emit_head(tc, pools, h, waves, qT_d, kT_d, v2_d, vp_d, o_d, consts, rec):
    nc = tc.nc
    wq, wk, wv, vppool, ppool, stpool, obpool, fpool = pools
    cst, a1out, crumb = consts

    qT = wq.tile([64, S], F32, tag="qT", name=f"qT{h}")
    kT = wk.tile([64, S], F32, tag="kT", name=f"kT{h}")
    v2 = wv.tile([128, NCHUNK * DA], F32, tag="v2", name=f"v2_{h}")
    vps = [vppool.tile([128, WAVE_CHUNKS * DA], F32, tag=f"vp{j}",
                       name=f"vp{j}h{h}") for j in range(2)]
    vp_dmas = []
    if "pe" in rec:
        labs = nc.gpsimd.tensor_copy(_crumb_cell2(rec, crumb),
                                     cst[0:1, 5:6])
        add_dep_helper(labs.ins, rec["pe"].ins, sync=True,
                       reason="Pool observes PE before head loads")
    rec["dmas"].append(nc.gpsimd.dma_start(out=qT, in_=qT_d[h]))
    rec["dmas"].append(nc.gpsimd.dma_start(out=kT, in_=kT_d[h]))
    rec["dmas"].append(nc.gpsimd.dma_start(out=v2, in_=v2_d[h]))

    started = set()
    ob_tiles = {}
    fin_due = []
    CB = WAVE_CHUNKS * BLK          # scratch corner base col in score tiles

    def _dummy_mm(dst, src_ap):
        mm = nc.tensor.matmul(dst, lhsT=src_ap, rhs=src_ap,
                              start=True, stop=True, skip_group_check=True)
        rec["pe"] = mm
        return mm

    def _crumb_cell():
        c = rec["ncr"]
        rec["ncr"] += 1
        assert rec["ncr"] <= 512
        return crumb[c // 512: c // 512 + 1, c % 512: c % 512 + 1]

    def _open_ob(bank, scorner):
        d1 = None
        if rec["om"] is not None:
            d1 = _dummy_mm(scorner, rec["om"])          # PE observes DVE
        ob = obpool.tile([128, 512], F32, tag="ob", name=f"obh{h}_{bank}")
        d2 = _dummy_mm(ob[0:1, 460:461], cst[0:1, 0:1])  # absorb PE drain
        if d1 is not None:
            add_dep_helper(d2.ins, d1.ins, sync=False,
                           reason="DVE observer before fresh-bank touch")
        ob_tiles[bank] = ob
        return ob

    def _finalize(bank):
        ob = ob_tiles.pop(bank)
        j0 = bank * OPB
        nq = min(OPB, NB - j0)
        gbank = len(rec["odma"])
        if gbank >= 3:
            # om slot recycles (bufs=4): bring DVE up to date with both
            # readers of the old tenant (out-DMA and the Pool crumb copy)
            dabs = nc.vector.tensor_copy(_crumb_cell(), cst[0:1, 2:3])
            add_dep_helper(dabs.ins, rec["odma"][gbank - 3].ins, sync=True,
                           reason="DVE observes om slot release")
            dabs2 = nc.vector.tensor_copy(_crumb_cell(), cst[0:1, 2:3])
            add_dep_helper(dabs2.ins, rec["pool_hist"][gbank - 3].ins,
                           sync=True, reason="DVE observes om Pool reader")
        om = fpool.tile([128, OPB * BLK], F32, tag="om", name=f"omh{h}_{bank}")
        rcp = fpool.tile([128, 1], F32, tag="rcp", name=f"rcph{h}_{bank}")
        for j in range(nq):
            nc.vector.reciprocal(
                rcp[0:64, :], ob[0:64, j * DA + D: j * DA + D + 1])
            rec["dve"] = nc.vector.tensor_scalar_mul(
                om[0:64, j * BLK: (j + 1) * BLK],
                ob[0:64, j * DA: j * DA + D],
                rcp[0:64, :])
        rec["pool"] = nc.gpsimd.tensor_copy(
            out=_crumb_cell(),
            in_=om[0:1, (nq - 1) * BLK: (nq - 1) * BLK + 1])
        rec["pool_hist"].append(rec["pool"])
        dma = nc.gpsimd.dma_start(
            out=o_d[h][bank][:, :].rearrange("(j p) c -> p j c", p=BLK),
            in_=om[0:64, : nq * BLK].rearrange("p (j c) -> p j c", c=BLK))
        rec["dmas"].append(dma)
        rec["odma"].append(dma)
        # last-written om region: a RAW dep on it covers every ob read above
        rec["om"] = om[0:1, (nq - 1) * BLK: (nq - 1) * BLK + 1]

    def emit_qk(wave, st):
        for ci, (i, ch, fst, lst, sidx) in enumerate(wave):
            c0 = ci * BLK
            rq = slice(i * BLK, (i + 1) * BLK)
            if ch[0] == "P":
                t = ch[1]
                rec["pe"] = nc.tensor.matmul(
                    st[:, c0: c0 + BLK],
                    lhsT=kT[0:64, 2 * t * BLK: (2 * t + 2) * BLK],
                    rhs=qT[0:64, rq],
                    start=True, stop=True, skip_group_check=True)
            else:
                for half, g in enumerate(ch[1:]):
                    if g is None:
                        g = 0     # filler: any finite scores; V half is zero
                    rec["pe"] = nc.tensor.matmul(
                        st[half * 64: half * 64 + 64, c0: c0 + BLK],
                        lhsT=kT[0:64, g * BLK: (g + 1) * BLK],
                        rhs=qT[0:64, rq],
                        start=True, stop=True, skip_group_check=True)

    def emit_pv(wave, pT, vp, w):
        cur = max(ob_tiles)
        # start=False: a start=True matmul clears the WHOLE bank's
        # has_written bits (hardware), which would wipe the in-flight
        # accumulation of a q-block split across waves in this bank
        mm = nc.tensor.matmul(
            ob_tiles[cur][0:1, 461:462], lhsT=pT[0:1, 0:1],
            rhs=pT[0:1, 0:1], start=False, stop=True, skip_group_check=True)
        rec["pe"] = mm
        svp = 0
        for ci, (i, ch, fst, lst, sidx) in enumerate(wave):
            c0 = ci * BLK
            bank = i // OPB
            if bank not in ob_tiles:
                prev_ob = ob_tiles[max(ob_tiles)]
                _open_ob(bank, prev_ob[0:1, 462:463])
            ob = ob_tiles[bank]
            osl = ob[0:64, (i % OPB) * DA: (i % OPB) * DA + DA]
            if ch[0] == "P":
                rhs = v2[:, ch[1] * DA: (ch[1] + 1) * DA]
            else:
                rhs = vp[:, svp * DA: (svp + 1) * DA]
                svp += 1
            rec["pe"] = nc.tensor.matmul(
                osl, lhsT=pT[:, c0: c0 + BLK], rhs=rhs,
                start=(i not in started), stop=lst, skip_group_check=True)
            started.add(i)
            if lst and (i == (bank + 1) * OPB - 1 or i == NB - 1):
                fin_due.append(bank)
        newest = max(ob_tiles)
        for bank in [b for b in fin_due if b != newest]:
            fin_due.remove(bank)
            _finalize(bank)

    # ---- head preamble: first score tile is the corner target for the
    # preamble observers (benign: these writes precede its exp)
    st0 = stpool.tile([128, WAVE_CHUNKS * BLK + BLK], F32, tag="st",
                      name=f"sth{h}_0")
    _dummy_mm(st0[0:1, CB: CB + 1], cst[0:1, 0:1])           # PE drain
    for di, src in enumerate((qT, kT, v2)):                  # DMA queues
        _dummy_mm(st0[0:1, CB + 1 + di: CB + 2 + di], src[0:1, 0:1])
    if rec["om"] is not None:                                # DVE (prev head)
        _dummy_mm(st0[0:1, CB + 4: CB + 5], rec["om"])
    _open_ob(0, st0[0:1, CB + 5: CB + 6])

    # ---- software-pipelined waves ----
    def load_vp(j, absorb):
        wave = waves[j]
        scnt = sum(1 for e in wave if e[1][0] == "S")
        s0 = min((e[4] for e in wave if e[4] >= 0), default=0)
        vp = vps[j % 2]
        pins = []
        if absorb:
            # buffer reused from wave j-2: Pool must observe the PV matmuls
            # that just finished reading it (rec["pe"] is exactly the last
            # one at this call site) and the DMA that wrote it
            pabs = nc.gpsimd.tensor_copy(_crumb_cell(), cst[0:1, 3:4])
            add_dep_helper(pabs.ins, rec["pe"].ins, sync=True,
                           reason="Pool observes vp buffer PV reads")
            pins.append(pabs)
            rel_dma = vp_dmas[j - 2]
            if rel_dma is not None:
                pabs2 = nc.gpsimd.tensor_copy(_crumb_cell(), cst[0:1, 4:5])
                add_dep_helper(pabs2.ins, rel_dma.ins, sync=True,
                               reason="Pool observes vp buffer old load")
                pins.append(pabs2)
        if scnt:
            vdma = nc.gpsimd.dma_start(
                out=vp[:, : scnt * DA].rearrange("p (s c) -> p s c", c=DA),
                in_=vp_d[h, s0: s0 + scnt].rearrange("s p c -> p s c"))
            for p in pins:
                add_dep_helper(vdma.ins, p.ins, sync=False,
                               reason="absorbers precede vp load")
            rec["dmas"].append(vdma)
        else:
            vdma = None
        assert len(vp_dmas) == j
        vp_dmas.append(vdma)

    load_vp(0, False)
    if len(waves) > 1:
        load_vp(1, False)
    prev = None
    for w, wave in enumerate(waves):
        if w == 0:
            st = st0
        else:
            st = stpool.tile([128, WAVE_CHUNKS * BLK + BLK], F32, tag="st",
                             name=f"sth{h}_{w}")
            _dummy_mm(st[0:1, CB: CB + 1], cst[0:1, 0:1])
        vp = vps[w % 2]
        emit_qk(wave, st)
        dc = _dummy_mm(st[0:1, CB + 6: CB + 7], cst[0:1, 0:1])
        a1 = nc.scalar.activation(
            out=a1out[0:1, 0:1], in_=st[0:1, CB + 6: CB + 7],
            func=mybir.ActivationFunctionType.Copy)
        add_dep_helper(a1.ins, dc.ins, sync=True,
                       reason="ACT observes PE after wave QK")
        rec["act"] = a1
        pT = ppool.tile([128, WAVE_CHUNKS * BLK], F32, tag="pT",
                        name=f"pTh{h}_{w}")
        ncols = len(wave) * BLK
        rec["act"] = nc.scalar.activation(
            out=pT[:, :ncols], in_=st[:, :ncols],
            func=mybir.ActivationFunctionType.Exp, scale=SCALE)
        if prev is not None:
            emit_pv(prev[1], prev[2], prev[3], prev[0])
            if w + 1 < len(waves):
                load_vp(w + 1, True)
        prev = (w, wave, pT, vp)
    emit_pv(prev[1], prev[2], prev[3], prev[0])
    for bank in list(fin_due):
        fin_due.remove(bank)
        _finalize(bank)
    for bank in sorted(ob_tiles):
        _finalize(bank)


def _build_program(bm: np.ndarray):
    import os as _os
    hpc = int(_os.environ.get("BB_HPC", HPC))
    nwaves = int(_os.environ.get("BB_NWAVES", 0))
    waves, ns = _build_schedule(bm)
    if nwaves:
        waves = waves[:nwaves]
    nc = bass.Bass("TRN2", target_bir_lowering=False, debug=False,
                   enable_asserts=False)
    qT_d = nc.dram_tensor("qT", [HPC, 64, S], F32, kind="ExternalInput")
    kT_d = nc.dram_tensor("kT", [HPC, 64, S], F32, kind="ExternalInput")
    v2_d = nc.dram_tensor("v2", [HPC, 128, NCHUNK * DA], F32,
                          kind="ExternalInput")
    vp_d = nc.dram_tensor("vp", [HPC, ns, 128, DA], F32, kind="ExternalInput")
    o_d = [[nc.dram_tensor(f"o_{hh}_{bb}",
                           [min(OPB, NB - bb * OPB) * BLK, D], F32,
                           kind="ExternalOutput")
            for bb in range(NBANK)] for hh in range(HPC)]

    with tile.TileContext(nc) as tc:
        with (
            tc.tile_pool(name="wq", bufs=HPC) as wq,
            tc.tile_pool(name="wk", bufs=HPC) as wk,
            tc.tile_pool(name="wv", bufs=HPC) as wv,
            tc.tile_pool(name="vpp", bufs=HPC) as vppool,
            tc.tile_pool(name="pT", bufs=2) as ppool,
            tc.tile_pool(name="st", bufs=2, space="PSUM") as stpool,
            tc.tile_pool(name="ob", bufs=2, space="PSUM") as obpool,
            tc.tile_pool(name="fin", bufs=3) as fpool,
            tc.tile_pool(name="cstp", bufs=1) as cpool,
        ):
            pools = (wq, wk, wv, vppool, ppool, stpool, obpool, fpool)
            cst = cpool.tile([128, 8], F32, tag="cst", name="cst")
            tc.nc.vector.memset(cst, 0.0)
            a1out = cpool.tile([128, 4], F32, tag="a1out", name="a1out")
            crumb = cpool.tile([128, 512], F32, tag="crumb", name="crumb")
            rec = {"dmas": [], "odma": [], "om": None, "ncr": 0,
                   "pool_hist": []}
            tc.nc.gpsimd.tensor_copy(_crumb_cell2(rec, crumb), cst[0:1, 6:7])
            for hh in range(hpc):
                _emit_head(tc, pools, hh, waves, qT_d, kT_d, v2_d, vp_d, o_d,
                           (cst, a1out, crumb), rec)
            # SP runs nothing; feed it one-wait nops covering each proc so
            # the framework's tail drain has no unobserved semaphores left
            tail = [rec[k] for k in ("pe", "act", "dve", "pool") if k in rec]
            tail += rec["dmas"][-16:]
            for td in tail:
                nop = tc.nc.sync.nop(nofuse=True)
                add_dep_helper(nop.ins, td.ins, sync=True,
                               reason="SP observes proc before tail drain")
    return nc


_CACHE = {}


def _get_program(bm: np.ndarray):
    key = bm.tobytes()
    if key not in _CACHE:
        _CACHE[key] = _build_program(bm)
    return _CACHE[key]


# -------------------------------------------------------------------- entry

def _prep_inputs(q, k, v, waves, ns):
    q = np.ascontiguousarray(np.asarray(q), dtype=np.float32)
    k = np.ascontiguousarray(np.asarray(k), dtype=np.float32)
    v = np.ascontiguousarray(np.asarray(v), dtype=np.float32)
    qT = np.ascontiguousarray(q.reshape(B * H, S, D).transpose(0, 2, 1))
    kT = np.ascontiguousarray(k.reshape(B * H, S, D).transpose(0, 2, 1))
    vA = np.concatenate(
        [v.reshape(B * H, S, D),
         np.ones((B * H, S, 1), dtype=np.float32)], axis=2)   # [24, S, 65]
    v2 = np.ascontiguousarray(
        vA.reshape(B * H, NCHUNK, 128, DA).transpose(0, 2, 1, 3)
        .reshape(B * H, 128, NCHUNK * DA))
    # gather V pairs for S chunks (absent halves stay zero)
    vp = np.zeros((B * H, ns, 128, DA), dtype=np.float32)
    vblk = vA.reshape(B * H, NB, BLK, DA)
    for wave in waves:
        for (i, ch, fst, lst, sidx) in wave:
            if ch[0] != "S":
                continue
            for half, g in enumerate(ch[1:]):
                if g is not None:
                    vp[:, sidx, half * 64: half * 64 + 64, :] = vblk[:, g]
    return qT, kT, v2, vp


def _run(inputs, trace=False):
    q, k, v, mask = inputs["q"], inputs["k"], inputs["v"], inputs["mask"]
    bm = _block_mask(mask)
    nc = _get_program(bm)
    waves, ns = _build_schedule(bm)
    qT, kT, v2, vp = _prep_inputs(q, k, v, waves, ns)
    in_maps = []
    for c in range(NCORES):
        sl = slice(c * HPC, (c + 1) * HPC)
        in_maps.append({
            "qT": np.ascontiguousarray(qT[sl]),
            "kT": np.ascontiguousarray(kT[sl]),
            "v2": np.ascontiguousarray(v2[sl]),
            "vp": np.ascontiguousarray(vp[sl]),
        })
    bkr = run_bass_kernel_spmd(nc, in_maps, list(range(NCORES)), trace=trace)
    pieces = []
    for r in bkr.results:
        for hh in range(HPC):
            pieces.append(np.concatenate(
                [np.asarray(r[f"o_{hh}_{bb}"]) for bb in range(NBANK)],
                axis=0))
    out = np.stack(pieces, axis=0).reshape(B, H, S, D).astype(np.float32)
    return out, bkr


def kernel(**inputs):
    out, _ = _run(inputs, trace=False)
    return out



# revision 8
# speedup vs baseline: 2.1238x; 2.1238x over previous
"""BigBird simulated attention on 8 Trainium2 NeuronCores.

Strategy
--------
B*H = 24 (batch, head) pairs are sharded 3-per-core across 8 cores (data/head
parallel). The BigBird mask is block-constant on 64x64 tiles, so the host
compresses it to a 64x64 block map and bakes a block-sparse schedule directly
into the instruction stream (the mask never goes to the device).

Per (head, q-block of 64 rows) scores are computed TRANSPOSED (S^T: k on
partitions, q on free) so the exp'd probabilities are directly the stationary
operand of the PV matmul -- no on-chip transposes:

  S^T[k, q] = sum_d K[k, d] Q[q, d]    (lhsT = K^T block cols, rhs = Q^T)
  P^T = exp(S^T / 8)                    (one ScalarE activation per wave)
  acc[q, :] = sum_k P^T[k, q]^T Vaug[k, :]    with Vaug = [V | 1]

The ones-column of Vaug makes acc[:, 64] the softmax denominator, so the
normalization is one reciprocal + per-partition-scalar multiply at the end.
Max-subtraction is skipped: scores are ~N(0,1) after scaling, exp cannot
overflow, and softmax is shift-invariant.

Attended k-blocks are packed two-per-"chunk" (one 128-partition score tile
column group). Aligned pairs (2t, 2t+1) come straight from a resident packed
V ([128, 32*65], natural 128-row chunks); arbitrary pairs use a host-gathered
per-chunk V-pair tensor streamed per wave. Every PV matmul is then a uniform
128-contraction. All QK matmuls keep lhsT in PE row group 0-1 (lower 64
partitions) and slice the output partition range via column tiling instead --
mixing lower- and upper-row-group LDWEIGHTS hangs the PE at runtime on this
toolchain (bisected empirically).

The many 1x1 "observer" matmuls / nops exist because every engine instruction
on this toolchain carries at most ONE hardware sync wait: each observer
brings one engine up to date with one foreign semaphore so no real
instruction ever needs two waits.
"""

import numpy as np

import concourse.bass as bass
import concourse.tile as tile
from concourse import mybir
from concourse.bass_utils import run_bass_kernel_spmd
from concourse.tile_rust import add_dep_helper

B, H, S, D = 2, 12, 4096, 64
BLK = 64
NB = S // BLK            # 64 blocks per axis
DA = D + 1               # v plus ones column
NCORES = 8
HPC = B * H // NCORES    # heads per core
SCALE = 1.0 / 8.0        # 1/sqrt(64)
WAVE_CHUNKS = 23         # 23*64 cols used of a 3-bank score tile; tail reserved
OPB = 7                  # q-blocks per psum output bank (7*65 = 455 <= 512)
NBANK = (NB + OPB - 1) // OPB
NCHUNK = S // 128        # natural 128-row chunks of V

F32 = mybir.dt.float32
BF16 = mybir.dt.bfloat16


# ----------------------------------------------------------------- schedule

def _block_mask(mask: np.ndarray) -> np.ndarray:
    m = np.asarray(mask).reshape(NB, BLK, NB, BLK)
    bm = m[:, 0, :, 0]
    assert bool(np.all(m == bm[:, None, :, None])), (
        "mask is not 64x64 block-constant; this kernel's schedule requires it"
    )
    return bm > 0


def _qblock_chunks(row: np.ndarray):
    """Chunks for one q-block; each chunk = 2 block slots (128 partitions).

    ('P', t, None): aligned pair (2t, 2t+1) -- V comes from the resident
        packed tensor.
    ('S', gA, gB): arbitrary pair (either may be None) -- V comes from the
        host-gathered per-chunk pair tensor.
    """
    L = set(np.nonzero(row)[0].tolist())
    chunks, singles = [], []
    for t in range(NB // 2):
        a, b = 2 * t, 2 * t + 1
        if a in L and b in L:
            chunks.append(("P", t, None))
        else:
            if a in L:
                singles.append(a)
            if b in L:
                singles.append(b)
    for k in range(0, len(singles), 2):
        gA = singles[k]
        gB = singles[k + 1] if k + 1 < len(singles) else None
        chunks.append(("S", gA, gB))
    cov = []
    for c in chunks:
        if c[0] == "P":
            cov += [2 * c[1], 2 * c[1] + 1]
        else:
            cov += [g for g in c[1:] if g is not None]
    assert sorted(cov) == sorted(L)
    return chunks


def _build_schedule(bm: np.ndarray):
    """Flat chunk list [(i, chunk, first_of_i, last_of_i, s_idx)] in waves.

    s_idx: running index into the packed S-chunk V-pair tensor (or -1).
    """
    flat = []
    ns = 0
    for i in range(NB):
        chunks = _qblock_chunks(bm[i])
        assert chunks, f"q-block {i} attends to nothing"
        for ci, ch in enumerate(chunks):
            sidx = -1
            if ch[0] == "S":
                sidx = ns
                ns += 1
            flat.append((i, ch, ci == 0, ci == len(chunks) - 1, sidx))
    waves = [flat[o: o + WAVE_CHUNKS] for o in range(0, len(flat), WAVE_CHUNKS)]
    return waves, ns


# ------------------------------------------------------------------ program

def _crumb_cell2(rec, crumb):
    c = rec["ncr"]
    rec["ncr"] += 1
    assert rec["ncr"] <= 512
    return crumb[c // 512: c // 512 + 1, c % 512: c % 512 + 1]


def _emit_head(tc, pools, h, waves, qT_d, kT_d, v2_d, vp_d, o_d, consts, rec):
    nc = tc.nc
    wq, wk, wv, vppool, ppool, stpool, obpool, fpool = pools
    cst, a1out, crumb = consts

    qT = wq.tile([64, S], BF16, tag="qT", name=f"qT{h}")
    kT = wk.tile([64, S], BF16, tag="kT", name=f"kT{h}")
    v2 = wv.tile([128, NCHUNK * DA], BF16, tag="v2", name=f"v2_{h}")
    vps = [vppool.tile([128, WAVE_CHUNKS * DA], BF16, tag=f"vp{j}",
                       name=f"vp{j}h{h}") for j in range(2)]
    vp_dmas = []
    if "pe" in rec:
        labs = nc.gpsimd.tensor_copy(_crumb_cell2(rec, crumb),
                                     cst[0:1, 5:6])
        add_dep_helper(labs.ins, rec["pe"].ins, sync=True,
                       reason="Pool observes PE before head loads")
    rec["dmas"].append(nc.gpsimd.dma_start(out=qT, in_=qT_d[h]))
    rec["dmas"].append(nc.gpsimd.dma_start(out=kT, in_=kT_d[h]))
    rec["dmas"].append(nc.gpsimd.dma_start(out=v2, in_=v2_d[h]))

    started = set()
    ob_tiles = {}
    fin_due = []
    CB = WAVE_CHUNKS * BLK          # scratch corner base col in score tiles

    def _dummy_mm(dst, src_ap):
        mm = nc.tensor.matmul(dst, lhsT=src_ap, rhs=src_ap,
                              start=True, stop=True, skip_group_check=True)
        rec["pe"] = mm
        return mm

    def _crumb_cell():
        c = rec["ncr"]
        rec["ncr"] += 1
        assert rec["ncr"] <= 512
        return crumb[c // 512: c // 512 + 1, c % 512: c % 512 + 1]

    def _open_ob(bank, scorner):
        d1 = None
        if rec["om"] is not None:
            d1 = _dummy_mm(scorner, rec["om"])          # PE observes DVE
        ob = obpool.tile([128, 512], F32, tag="ob", name=f"obh{h}_{bank}")
        d2 = _dummy_mm(ob[0:1, 460:461], cst[0:1, 0:1])  # absorb PE drain
        if d1 is not None:
            add_dep_helper(d2.ins, d1.ins, sync=False,
                           reason="DVE observer before fresh-bank touch")
        ob_tiles[bank] = ob
        return ob

    def _finalize(bank):
        ob = ob_tiles.pop(bank)
        j0 = bank * OPB
        nq = min(OPB, NB - j0)
        gbank = len(rec["odma"])
        if gbank >= 3:
            # om slot recycles (bufs=4): bring DVE up to date with both
            # readers of the old tenant (out-DMA and the Pool crumb copy)
            dabs = nc.vector.tensor_copy(_crumb_cell(), cst[0:1, 2:3])
            add_dep_helper(dabs.ins, rec["odma"][gbank - 3].ins, sync=True,
                           reason="DVE observes om slot release")
            dabs2 = nc.vector.tensor_copy(_crumb_cell(), cst[0:1, 2:3])
            add_dep_helper(dabs2.ins, rec["pool_hist"][gbank - 3].ins,
                           sync=True, reason="DVE observes om Pool reader")
        om = fpool.tile([128, OPB * BLK], F32, tag="om", name=f"omh{h}_{bank}")
        rcp = fpool.tile([128, 1], F32, tag="rcp", name=f"rcph{h}_{bank}")
        for j in range(nq):
            nc.vector.reciprocal(
                rcp[0:64, :], ob[0:64, j * DA + D: j * DA + D + 1])
            rec["dve"] = nc.vector.tensor_scalar_mul(
                om[0:64, j * BLK: (j + 1) * BLK],
                ob[0:64, j * DA: j * DA + D],
                rcp[0:64, :])
        rec["pool"] = nc.gpsimd.tensor_copy(
            out=_crumb_cell(),
            in_=om[0:1, (nq - 1) * BLK: (nq - 1) * BLK + 1])
        rec["pool_hist"].append(rec["pool"])
        dma = nc.gpsimd.dma_start(
            out=o_d[h][bank][:, :].rearrange("(j p) c -> p j c", p=BLK),
            in_=om[0:64, : nq * BLK].rearrange("p (j c) -> p j c", c=BLK))
        rec["dmas"].append(dma)
        rec["odma"].append(dma)
        # last-written om region: a RAW dep on it covers every ob read above
        rec["om"] = om[0:1, (nq - 1) * BLK: (nq - 1) * BLK + 1]

    def emit_qk(wave, st):
        for ci, (i, ch, fst, lst, sidx) in enumerate(wave):
            c0 = ci * BLK
            rq = slice(i * BLK, (i + 1) * BLK)
            if ch[0] == "P":
                t = ch[1]
                rec["pe"] = nc.tensor.matmul(
                    st[:, c0: c0 + BLK],
                    lhsT=kT[0:64, 2 * t * BLK: (2 * t + 2) * BLK],
                    rhs=qT[0:64, rq],
                    start=True, stop=True, skip_group_check=True)
            else:
                for half, g in enumerate(ch[1:]):
                    if g is None:
                        g = 0     # filler: any finite scores; V half is zero
                    rec["pe"] = nc.tensor.matmul(
                        st[half * 64: half * 64 + 64, c0: c0 + BLK],
                        lhsT=kT[0:64, g * BLK: (g + 1) * BLK],
                        rhs=qT[0:64, rq],
                        start=True, stop=True, skip_group_check=True)

    def emit_pv(wave, pT, vp, w):
        cur = max(ob_tiles)
        # start=False: a start=True matmul clears the WHOLE bank's
        # has_written bits (hardware), which would wipe the in-flight
        # accumulation of a q-block split across waves in this bank
        mm = nc.tensor.matmul(
            ob_tiles[cur][0:1, 461:462], lhsT=pT[0:1, 0:1],
            rhs=pT[0:1, 0:1], start=False, stop=True, skip_group_check=True)
        rec["pe"] = mm
        svp = 0
        for ci, (i, ch, fst, lst, sidx) in enumerate(wave):
            c0 = ci * BLK
            bank = i // OPB
            if bank not in ob_tiles:
                prev_ob = ob_tiles[max(ob_tiles)]
                _open_ob(bank, prev_ob[0:1, 462:463])
            ob = ob_tiles[bank]
            osl = ob[0:64, (i % OPB) * DA: (i % OPB) * DA + DA]
            if ch[0] == "P":
                rhs = v2[:, ch[1] * DA: (ch[1] + 1) * DA]
            else:
                rhs = vp[:, svp * DA: (svp + 1) * DA]
                svp += 1
            rec["pe"] = nc.tensor.matmul(
                osl, lhsT=pT[:, c0: c0 + BLK], rhs=rhs,
                start=(i not in started), stop=lst, skip_group_check=True)
            started.add(i)
            if lst and (i == (bank + 1) * OPB - 1 or i == NB - 1):
                fin_due.append(bank)
        newest = max(ob_tiles)
        for bank in [b for b in fin_due if b != newest]:
            fin_due.remove(bank)
            _finalize(bank)

    # ---- head preamble: first score tile is the corner target for the
    # preamble observers (benign: these writes precede its exp)
    st0 = stpool.tile([128, WAVE_CHUNKS * BLK + BLK], F32, tag="st",
                      name=f"sth{h}_0")
    _dummy_mm(st0[0:1, CB: CB + 1], cst[0:1, 0:1])           # PE drain
    for di, src in enumerate((qT, kT, v2)):                  # DMA queues
        _dummy_mm(st0[0:1, CB + 1 + di: CB + 2 + di], src[0:1, 0:1])
    if rec["om"] is not None:                                # DVE (prev head)
        _dummy_mm(st0[0:1, CB + 4: CB + 5], rec["om"])
    _open_ob(0, st0[0:1, CB + 5: CB + 6])

    # ---- software-pipelined waves ----
    def load_vp(j, absorb):
        wave = waves[j]
        scnt = sum(1 for e in wave if e[1][0] == "S")
        s0 = min((e[4] for e in wave if e[4] >= 0), default=0)
        vp = vps[j % 2]
        pins = []
        if absorb:
            # buffer reused from wave j-2: Pool must observe the PV matmuls
            # that just finished reading it (rec["pe"] is exactly the last
            # one at this call site) and the DMA that wrote it
            pabs = nc.gpsimd.tensor_copy(_crumb_cell(), cst[0:1, 3:4])
            add_dep_helper(pabs.ins, rec["pe"].ins, sync=True,
                           reason="Pool observes vp buffer PV reads")
            pins.append(pabs)
            rel_dma = vp_dmas[j - 2]
            if rel_dma is not None:
                pabs2 = nc.gpsimd.tensor_copy(_crumb_cell(), cst[0:1, 4:5])
                add_dep_helper(pabs2.ins, rel_dma.ins, sync=True,
                               reason="Pool observes vp buffer old load")
                pins.append(pabs2)
        if scnt:
            vdma = nc.gpsimd.dma_start(
                out=vp[:, : scnt * DA].rearrange("p (s c) -> p s c", c=DA),
                in_=vp_d[h, s0: s0 + scnt].rearrange("s p c -> p s c"))
            for p in pins:
                add_dep_helper(vdma.ins, p.ins, sync=False,
                               reason="absorbers precede vp load")
            rec["dmas"].append(vdma)
        else:
            vdma = None
        assert len(vp_dmas) == j
        vp_dmas.append(vdma)

    load_vp(0, False)
    if len(waves) > 1:
        load_vp(1, False)
    prev = None
    for w, wave in enumerate(waves):
        if w == 0:
            st = st0
        else:
            st = stpool.tile([128, WAVE_CHUNKS * BLK + BLK], F32, tag="st",
                             name=f"sth{h}_{w}")
            _dummy_mm(st[0:1, CB: CB + 1], cst[0:1, 0:1])
        vp = vps[w % 2]
        emit_qk(wave, st)
        dc = _dummy_mm(st[0:1, CB + 6: CB + 7], cst[0:1, 0:1])
        a1 = nc.scalar.activation(
            out=a1out[0:1, 0:1], in_=st[0:1, CB + 6: CB + 7],
            func=mybir.ActivationFunctionType.Copy)
        add_dep_helper(a1.ins, dc.ins, sync=True,
                       reason="ACT observes PE after wave QK")
        rec["act"] = a1
        pT = ppool.tile([128, WAVE_CHUNKS * BLK], BF16, tag="pT",
                        name=f"pTh{h}_{w}")
        ncols = len(wave) * BLK
        rec["act"] = nc.scalar.activation(
            out=pT[:, :ncols], in_=st[:, :ncols],
            func=mybir.ActivationFunctionType.Exp, scale=SCALE)
        if prev is not None:
            emit_pv(prev[1], prev[2], prev[3], prev[0])
            if w + 1 < len(waves):
                load_vp(w + 1, True)
        prev = (w, wave, pT, vp)
    emit_pv(prev[1], prev[2], prev[3], prev[0])
    for bank in list(fin_due):
        fin_due.remove(bank)
        _finalize(bank)
    for bank in sorted(ob_tiles):
        _finalize(bank)


def _build_program(bm: np.ndarray):
    import os as _os
    hpc = int(_os.environ.get("BB_HPC", HPC))
    nwaves = int(_os.environ.get("BB_NWAVES", 0))
    waves, ns = _build_schedule(bm)
    if nwaves:
        waves = waves[:nwaves]
    nc = bass.Bass("TRN2", target_bir_lowering=False, debug=False,
                   enable_asserts=False)
    qT_d = nc.dram_tensor("qT", [HPC, 64, S], BF16, kind="ExternalInput")
    kT_d = nc.dram_tensor("kT", [HPC, 64, S], BF16, kind="ExternalInput")
    v2_d = nc.dram_tensor("v2", [HPC, 128, NCHUNK * DA], BF16,
                          kind="ExternalInput")
    vp_d = nc.dram_tensor("vp", [HPC, ns, 128, DA], BF16,
                          kind="ExternalInput")
    o_d = [[nc.dram_tensor(f"o_{hh}_{bb}",
                           [min(OPB, NB - bb * OPB) * BLK, D], F32,
                           kind="ExternalOutput")
            for bb in range(NBANK)] for hh in range(HPC)]

    with tile.TileContext(nc) as tc:
        with (
            tc.tile_pool(name="wq", bufs=HPC) as wq,
            tc.tile_pool(name="wk", bufs=HPC) as wk,
            tc.tile_pool(name="wv", bufs=HPC) as wv,
            tc.tile_pool(name="vpp", bufs=HPC) as vppool,
            tc.tile_pool(name="pT", bufs=3) as ppool,
            tc.tile_pool(name="st", bufs=2, space="PSUM") as stpool,
            tc.tile_pool(name="ob", bufs=2, space="PSUM") as obpool,
            tc.tile_pool(name="fin", bufs=3) as fpool,
            tc.tile_pool(name="cstp", bufs=1) as cpool,
        ):
            pools = (wq, wk, wv, vppool, ppool, stpool, obpool, fpool)
            cst = cpool.tile([128, 8], F32, tag="cst", name="cst")
            tc.nc.vector.memset(cst, 0.0)
            a1out = cpool.tile([128, 4], F32, tag="a1out", name="a1out")
            crumb = cpool.tile([128, 512], F32, tag="crumb", name="crumb")
            rec = {"dmas": [], "odma": [], "om": None, "ncr": 0,
                   "pool_hist": []}
            tc.nc.gpsimd.tensor_copy(_crumb_cell2(rec, crumb), cst[0:1, 6:7])
            for hh in range(hpc):
                _emit_head(tc, pools, hh, waves, qT_d, kT_d, v2_d, vp_d, o_d,
                           (cst, a1out, crumb), rec)
            # SP runs nothing; feed it one-wait nops covering each proc so
            # the framework's tail drain has no unobserved semaphores left
            tail = [rec[k] for k in ("pe", "act", "dve", "pool") if k in rec]
            tail += rec["dmas"][-16:]
            for td in tail:
                nop = tc.nc.sync.nop(nofuse=True)
                add_dep_helper(nop.ins, td.ins, sync=True,
                               reason="SP observes proc before tail drain")
    # bf16 matmuls are split into Ldweights+Matmult, which lets the
    # scheduler reorder the PE stream past the hand-placed observers; run
    # the framework passes that re-establish the 1-wait-per-instruction
    # hardware constraint (Bacc.compile does the same).
    import bass_rust as _bass_rust
    _bass_rust.move_matmul_waits_to_ldweights(nc.m)
    _bass_rust.generate_event_semaphores(nc)
    return nc


_CACHE = {}


def _get_program(bm: np.ndarray):
    key = bm.tobytes()
    if key not in _CACHE:
        _CACHE[key] = _build_program(bm)
    return _CACHE[key]


# -------------------------------------------------------------------- entry

def _prep_inputs(q, k, v, waves, ns):
    import ml_dtypes
    bf16 = ml_dtypes.bfloat16
    q = np.ascontiguousarray(np.asarray(q), dtype=np.float32)
    k = np.ascontiguousarray(np.asarray(k), dtype=np.float32)
    v = np.ascontiguousarray(np.asarray(v), dtype=np.float32)
    qT = np.ascontiguousarray(
        q.reshape(B * H, S, D).transpose(0, 2, 1).astype(bf16))
    kT = np.ascontiguousarray(
        k.reshape(B * H, S, D).transpose(0, 2, 1).astype(bf16))
    vA = np.concatenate(
        [v.reshape(B * H, S, D),
         np.ones((B * H, S, 1), dtype=np.float32)], axis=2).astype(bf16)
    v2 = np.ascontiguousarray(
        vA.reshape(B * H, NCHUNK, 128, DA).transpose(0, 2, 1, 3)
        .reshape(B * H, 128, NCHUNK * DA))
    # gather V pairs for S chunks (absent halves stay zero)
    vp = np.zeros((B * H, ns, 128, DA), dtype=bf16)
    vblk = vA.reshape(B * H, NB, BLK, DA)
    for wave in waves:
        for (i, ch, fst, lst, sidx) in wave:
            if ch[0] != "S":
                continue
            for half, g in enumerate(ch[1:]):
                if g is not None:
                    vp[:, sidx, half * 64: half * 64 + 64, :] = vblk[:, g]
    return qT, kT, v2, vp


def _run(inputs, trace=False):
    q, k, v, mask = inputs["q"], inputs["k"], inputs["v"], inputs["mask"]
    bm = _block_mask(mask)
    nc = _get_program(bm)
    waves, ns = _build_schedule(bm)
    qT, kT, v2, vp = _prep_inputs(q, k, v, waves, ns)
    in_maps = []
    for c in range(NCORES):
        sl = slice(c * HPC, (c + 1) * HPC)
        in_maps.append({
            "qT": np.ascontiguousarray(qT[sl]),
            "kT": np.ascontiguousarray(kT[sl]),
            "v2": np.ascontiguousarray(v2[sl]),
            "vp": np.ascontiguousarray(vp[sl]),
        })
    bkr = run_bass_kernel_spmd(nc, in_maps, list(range(NCORES)), trace=trace)
    pieces = []
    for r in bkr.results:
        for hh in range(HPC):
            pieces.append(np.concatenate(
                [np.asarray(r[f"o_{hh}_{bb}"]) for bb in range(NBANK)],
                axis=0))
    out = np.stack(pieces, axis=0).reshape(B, H, S, D).astype(np.float32)
    return out, bkr


def kernel(**inputs):
    out, _ = _run(inputs, trace=False)
    return out



# revision 10
# speedup vs baseline: 2.5558x; 1.2034x over previous
"""BigBird simulated attention on 8 Trainium2 NeuronCores.

Strategy
--------
B*H = 24 (batch, head) pairs are sharded 3-per-core across 8 cores (data/head
parallel). The BigBird mask is block-constant on 64x64 tiles, so the host
compresses it to a 64x64 block map and bakes a block-sparse schedule directly
into the instruction stream (the mask never goes to the device).

Per (head, q-block of 64 rows) scores are computed TRANSPOSED (S^T: k on
partitions, q on free) so the exp'd probabilities are directly the stationary
operand of the PV matmul -- no on-chip transposes:

  S^T[k, q] = sum_d K[k, d] Q[q, d]    (lhsT = K^T block cols, rhs = Q^T)
  P^T = exp(S^T / 8)                    (one ScalarE activation per wave)
  acc[q, :] = sum_k P^T[k, q]^T Vaug[k, :]    with Vaug = [V | 1]

All matmuls run in bf16 (tolerance is 2e-2; bf16 keeps rel err ~6e-3) which
is 4x the fp32 rate on the PE.

The ones-column of Vaug makes acc[:, 64] the softmax denominator, so the
normalization is one reciprocal + per-partition-scalar multiply at the end.
Max-subtraction is skipped: scores are ~N(0,1) after scaling, exp cannot
overflow, and softmax is shift-invariant.

Attended k-blocks are packed two-per-"chunk" (one 128-partition score tile
column group). Aligned pairs (2t, 2t+1) come from a resident packed V
([128, 33*65]); the globally-shared pair (0, 63) is appended host-side as
resident pair #32 (and as kT cols 4096..4224), turning what would be ~62
gathered chunks per head into plain resident chunks. Remaining arbitrary
pairs use a host-gathered per-chunk V-pair tensor streamed per wave. Every
PV matmul is then a uniform 128-contraction.

Sync: the Tile framework tracks all deps; after emission we run the Bacc
passes move_matmul_waits_to_ldweights + generate_event_semaphores, which
re-establish the TRN2 "at most one sync wait per instruction" constraint
(walrus refuses multi-wait instructions otherwise).
"""

import numpy as np

import concourse.bass as bass
import concourse.tile as tile
from concourse import mybir
from concourse.bass_utils import run_bass_kernel_spmd

B, H, S, D = 2, 12, 4096, 64
BLK = 64
NB = S // BLK            # 64 blocks per axis
DA = D + 1               # v plus ones column
NCORES = 8
HPC = B * H // NCORES    # heads per core
SCALE = 1.0 / 8.0        # 1/sqrt(64)
WAVE_CHUNKS = 24         # 24*64 cols = exactly 3 PSUM banks per score tile
OPB = 7                  # q-blocks per psum output bank (7*65 = 455 <= 512)
NBANK = (NB + OPB - 1) // OPB
NCHUNK = S // 128        # natural 128-row chunks of V
PAIR_G = NCHUNK          # resident pair index for the global (0, 63) pair
NPAIR = NCHUNK + 1

F32 = mybir.dt.float32
BF16 = mybir.dt.bfloat16


# ----------------------------------------------------------------- schedule

def _block_mask(mask: np.ndarray) -> np.ndarray:
    m = np.asarray(mask).reshape(NB, BLK, NB, BLK)
    bm = m[:, 0, :, 0]
    assert bool(np.all(m == bm[:, None, :, None])), (
        "mask is not 64x64 block-constant; this kernel's schedule requires it"
    )
    return bm > 0


def _qblock_chunks(row: np.ndarray):
    """Chunks for one q-block; each chunk = 2 block slots (128 partitions).

    ('P', t, None): resident pair t -- aligned (2t, 2t+1) for t < 32, or the
        appended global pair (0, 63) for t == PAIR_G.
    ('S', gA, gB): arbitrary pair (either may be None) -- V comes from the
        host-gathered per-chunk pair tensor.
    """
    L = set(np.nonzero(row)[0].tolist())
    chunks, singles = [], []
    if len(L) < NB and 0 in L and NB - 1 in L:
        # global columns pair, resident as pair PAIR_G
        L.discard(0)
        L.discard(NB - 1)
        chunks.append(("P", PAIR_G, None))
    for t in range(NB // 2):
        a, b = 2 * t, 2 * t + 1
        if a in L and b in L:
            chunks.append(("P", t, None))
        else:
            if a in L:
                singles.append(a)
            if b in L:
                singles.append(b)
    for k in range(0, len(singles), 2):
        gA = singles[k]
        gB = singles[k + 1] if k + 1 < len(singles) else None
        chunks.append(("S", gA, gB))
    assert chunks
    return chunks


def _build_schedule(bm: np.ndarray):
    """Flat chunk list [(i, chunk, first_of_i, last_of_i, s_idx)] in waves.

    s_idx: running index into the packed S-chunk V-pair tensor (or -1).
    """
    flat = []
    ns = 0
    for i in range(NB):
        chunks = _qblock_chunks(bm[i])
        for ci, ch in enumerate(chunks):
            sidx = -1
            if ch[0] == "S":
                sidx = ns
                ns += 1
            flat.append((i, ch, ci == 0, ci == len(chunks) - 1, sidx))
    waves = [flat[o: o + WAVE_CHUNKS] for o in range(0, len(flat), WAVE_CHUNKS)]
    return waves, ns


# ------------------------------------------------------------------ program

def _emit_head(tc, pools, h, waves, qT_d, kT_d, v2_d, vp_d, o_d):
    nc = tc.nc
    wq, wk, wv, vppool, ppool, stpool, obpool, fpool = pools

    qT = wq.tile([64, S], BF16, tag="qT", name=f"qT{h}")
    kT = wk.tile([64, S + 2 * BLK], BF16, tag="kT", name=f"kT{h}")
    v2 = wv.tile([128, NPAIR * DA], BF16, tag="v2", name=f"v2_{h}")
    vps = [vppool.tile([128, WAVE_CHUNKS * DA], BF16, tag=f"vp{j}",
                       name=f"vp{j}h{h}") for j in range(2)]
    nc.gpsimd.dma_start(out=qT, in_=qT_d[h])
    nc.gpsimd.dma_start(out=kT, in_=kT_d[h])
    nc.gpsimd.dma_start(out=v2, in_=v2_d[h])

    started = set()
    ob_tiles = {}
    fin_due = []

    def _finalize(bank):
        ob = ob_tiles.pop(bank)
        j0 = bank * OPB
        nq = min(OPB, NB - j0)
        om = fpool.tile([128, OPB * BLK], F32, tag="om", name=f"omh{h}_{bank}")
        rcp = fpool.tile([128, 1], F32, tag="rcp", name=f"rcph{h}_{bank}")
        for j in range(nq):
            nc.vector.reciprocal(
                rcp[0:64, :], ob[0:64, j * DA + D: j * DA + D + 1])
            nc.vector.tensor_scalar_mul(
                om[0:64, j * BLK: (j + 1) * BLK],
                ob[0:64, j * DA: j * DA + D],
                rcp[0:64, :])
        nc.gpsimd.dma_start(
            out=o_d[h][bank][:, :].rearrange("(j p) c -> p j c", p=BLK),
            in_=om[0:64, : nq * BLK].rearrange("p (j c) -> p j c", c=BLK))

    def emit_qk(wave, st):
        for ci, (i, ch, fst, lst, sidx) in enumerate(wave):
            c0 = ci * BLK
            rq = slice(i * BLK, (i + 1) * BLK)
            if ch[0] == "P":
                t = ch[1]
                nc.tensor.matmul(
                    st[:, c0: c0 + BLK],
                    lhsT=kT[0:64, 2 * t * BLK: (2 * t + 2) * BLK],
                    rhs=qT[0:64, rq],
                    start=True, stop=True, skip_group_check=True)
            else:
                for half, g in enumerate(ch[1:]):
                    if g is None:
                        g = 0     # filler: any finite scores; V half is zero
                    nc.tensor.matmul(
                        st[half * 64: half * 64 + 64, c0: c0 + BLK],
                        lhsT=kT[0:64, g * BLK: (g + 1) * BLK],
                        rhs=qT[0:64, rq],
                        start=True, stop=True, skip_group_check=True)

    def emit_pv(wave, pT, vp):
        svp = 0
        for ci, (i, ch, fst, lst, sidx) in enumerate(wave):
            c0 = ci * BLK
            bank = i // OPB
            if bank not in ob_tiles:
                ob_tiles[bank] = obpool.tile([128, 512], F32, tag="ob",
                                             name=f"obh{h}_{bank}")
            ob = ob_tiles[bank]
            osl = ob[0:64, (i % OPB) * DA: (i % OPB) * DA + DA]
            if ch[0] == "P":
                rhs = v2[:, ch[1] * DA: (ch[1] + 1) * DA]
            else:
                rhs = vp[:, svp * DA: (svp + 1) * DA]
                svp += 1
            # start=True on a q-block's first chunk clears the whole bank's
            # has_written bits; safe because chunks of one q-block are
            # contiguous in the flat schedule, so no other q-block in this
            # bank has an accumulation in flight (data of FINISHED q-blocks
            # survives -- only the bits are cleared).
            nc.tensor.matmul(
                osl, lhsT=pT[:, c0: c0 + BLK], rhs=rhs,
                start=(i not in started), stop=lst, skip_group_check=True)
            started.add(i)
            if lst and (i == (bank + 1) * OPB - 1 or i == NB - 1):
                fin_due.append(bank)
        newest = max(ob_tiles)
        for bank in [b for b in fin_due if b != newest]:
            fin_due.remove(bank)
            _finalize(bank)

    def load_vp(j):
        wave = waves[j]
        scnt = sum(1 for e in wave if e[1][0] == "S")
        if not scnt:
            return
        s0 = min(e[4] for e in wave if e[4] >= 0)
        vp = vps[j % 2]
        nc.gpsimd.dma_start(
            out=vp[:, : scnt * DA].rearrange("p (s c) -> p s c", c=DA),
            in_=vp_d[h, s0: s0 + scnt].rearrange("s p c -> p s c"))

    load_vp(0)
    if len(waves) > 1:
        load_vp(1)
    prev = None
    for w, wave in enumerate(waves):
        st = stpool.tile([128, WAVE_CHUNKS * BLK], F32, tag="st",
                         name=f"sth{h}_{w}")
        emit_qk(wave, st)
        pT = ppool.tile([128, WAVE_CHUNKS * BLK], BF16, tag="pT",
                        name=f"pTh{h}_{w}")
        ncols = len(wave) * BLK
        nc.scalar.activation(
            out=pT[:, :ncols], in_=st[:, :ncols],
            func=mybir.ActivationFunctionType.Exp, scale=SCALE)
        if prev is not None:
            emit_pv(*prev)
            if w + 1 < len(waves):
                load_vp(w + 1)
        prev = (wave, pT, vps[w % 2])
    emit_pv(*prev)
    for bank in list(fin_due):
        fin_due.remove(bank)
        _finalize(bank)
    for bank in sorted(ob_tiles):
        _finalize(bank)


def _build_program(bm: np.ndarray):
    import os as _os
    hpc = int(_os.environ.get("BB_HPC", HPC))
    waves, ns = _build_schedule(bm)
    nc = bass.Bass("TRN2", target_bir_lowering=False, debug=False,
                   enable_asserts=False)
    qT_d = nc.dram_tensor("qT", [HPC, 64, S], BF16, kind="ExternalInput")
    kT_d = nc.dram_tensor("kT", [HPC, 64, S + 2 * BLK], BF16,
                          kind="ExternalInput")
    v2_d = nc.dram_tensor("v2", [HPC, 128, NPAIR * DA], BF16,
                          kind="ExternalInput")
    vp_d = nc.dram_tensor("vp", [HPC, max(ns, 1), 128, DA], BF16,
                          kind="ExternalInput")
    o_d = [[nc.dram_tensor(f"o_{hh}_{bb}",
                           [min(OPB, NB - bb * OPB) * BLK, D], F32,
                           kind="ExternalOutput")
            for bb in range(NBANK)] for hh in range(HPC)]

    with tile.TileContext(nc) as tc:
        with (
            tc.tile_pool(name="wq", bufs=HPC) as wq,
            tc.tile_pool(name="wk", bufs=HPC) as wk,
            tc.tile_pool(name="wv", bufs=HPC) as wv,
            tc.tile_pool(name="vpp", bufs=HPC) as vppool,
            tc.tile_pool(name="pT", bufs=3) as ppool,
            tc.tile_pool(name="st", bufs=2, space="PSUM") as stpool,
            tc.tile_pool(name="ob", bufs=2, space="PSUM") as obpool,
            tc.tile_pool(name="fin", bufs=3) as fpool,
        ):
            pools = (wq, wk, wv, vppool, ppool, stpool, obpool, fpool)
            for hh in range(hpc):
                _emit_head(tc, pools, hh, waves, qT_d, kT_d, v2_d, vp_d, o_d)
    # bf16 matmuls are split into Ldweights+Matmult, which lets the
    # scheduler interleave the PE stream; these framework passes then
    # re-establish the 1-sync-wait-per-instruction hardware constraint.
    import bass_rust as _bass_rust
    _bass_rust.move_matmul_waits_to_ldweights(nc.m)
    _bass_rust.generate_event_semaphores(nc)
    return nc


_CACHE = {}


def _get_program(bm: np.ndarray):
    key = bm.tobytes()
    if key not in _CACHE:
        _CACHE[key] = _build_program(bm)
    return _CACHE[key]


# -------------------------------------------------------------------- entry

def _prep_inputs(q, k, v, waves, ns):
    import ml_dtypes
    bf16 = ml_dtypes.bfloat16
    q = np.ascontiguousarray(np.asarray(q), dtype=np.float32)
    k = np.ascontiguousarray(np.asarray(k), dtype=np.float32)
    v = np.ascontiguousarray(np.asarray(v), dtype=np.float32)
    qT = np.ascontiguousarray(
        q.reshape(B * H, S, D).transpose(0, 2, 1).astype(bf16))
    kT = np.ascontiguousarray(
        k.reshape(B * H, S, D).transpose(0, 2, 1).astype(bf16))
    # extend kT with the (0, 63) block pair at cols S..S+128
    kTe = np.concatenate(
        [kT, kT[:, :, :BLK], kT[:, :, (NB - 1) * BLK:]], axis=2)
    vA = np.concatenate(
        [v.reshape(B * H, S, D),
         np.ones((B * H, S, 1), dtype=np.float32)], axis=2).astype(bf16)
    v2 = vA.reshape(B * H, NCHUNK, 128, DA)
    vblk = vA.reshape(B * H, NB, BLK, DA)
    pair_g = np.concatenate([vblk[:, 0], vblk[:, NB - 1]], axis=1)  # [24,128,DA]
    v2e = np.ascontiguousarray(
        np.concatenate([v2, pair_g[:, None]], axis=1)
        .transpose(0, 2, 1, 3).reshape(B * H, 128, NPAIR * DA))
    # gather V pairs for S chunks (absent halves stay zero)
    vp = np.zeros((B * H, max(ns, 1), 128, DA), dtype=bf16)
    for wave in waves:
        for (i, ch, fst, lst, sidx) in wave:
            if ch[0] != "S":
                continue
            for half, g in enumerate(ch[1:]):
                if g is not None:
                    vp[:, sidx, half * 64: half * 64 + 64, :] = vblk[:, g]
    return qT, kTe, v2e, vp


def _run(inputs, trace=False):
    q, k, v, mask = inputs["q"], inputs["k"], inputs["v"], inputs["mask"]
    bm = _block_mask(mask)
    nc = _get_program(bm)
    waves, ns = _build_schedule(bm)
    qT, kTe, v2e, vp = _prep_inputs(q, k, v, waves, ns)
    in_maps = []
    for c in range(NCORES):
        sl = slice(c * HPC, (c + 1) * HPC)
        in_maps.append({
            "qT": np.ascontiguousarray(qT[sl]),
            "kT": np.ascontiguousarray(kTe[sl]),
            "v2": np.ascontiguousarray(v2e[sl]),
            "vp": np.ascontiguousarray(vp[sl]),
        })
    bkr = run_bass_kernel_spmd(nc, in_maps, list(range(NCORES)), trace=trace)
    pieces = []
    for r in bkr.results:
        for hh in range(HPC):
            pieces.append(np.concatenate(
                [np.asarray(r[f"o_{hh}_{bb}"]) for bb in range(NBANK)],
                axis=0))
    out = np.stack(pieces, axis=0).reshape(B, H, S, D).astype(np.float32)
    return out, bkr


def kernel(**inputs):
    out, _ = _run(inputs, trace=False)
    return out
